# revision 2
# baseline (speedup 1.0000x reference)
"""Trainium2 Bass kernel for nn_BranchedNetwork (moe_routing).

Computation (reference):
    meas_embs = measurements @ W_meas + b_meas           [B, 512]
    embs      = concat([img_embs, meas_embs], axis=1)    [B, 1024]
    h_e       = relu(embs @ W1[e] + b1[e])               per expert e
    out_e     = h_e @ W2[e] + b2[e]
    p[i]      = out[command[i], i, 0]
    angle     = sigmoid(p) * 50 ; speed = clip(p, -1, 1)

Strategy:
  * Per-sample routing on the host: samples grouped by command id, each
    group padded to a multiple of 8*128 rows and split evenly over the
    8 cores (data parallel, weights replicated; per-expert tile counts
    identical on every core so one SPMD program serves all 8).
  * Only the selected expert runs per sample (4x less compute), and
    only column 0 of W2 is needed.
  * The measurement path is folded on the host:
      h_pre = img @ W1[e][:512] + measAug @ WfAug[e]
    with measAug = [meas; 1] (K=9) so the device contraction is
    K = 512 (img) + 9.
  * |w2[:, 0]| is folded into the layer-1 weights with hidden columns
    permuted by sign of w2, so layer 2 reduces to
    p = sum(relu(pos cols)) - sum(relu(neg cols)) + b2, computed for
    free by ACT/DVE accumulators during the relu pass.
  * Device per 128-row tile: a packed K=9 meas matmul (4 tiles run
    concurrently in separate PE row-groups via tile_position) + 4
    K=128 img matmuls accumulate psum [128 rows, 512 hid]; ACT does
    relu+accum on the positive columns, DVE on the negative ones.
  * Schedule: few large DMAs (A weights as 2, img as ~7 per-tile-group
    chunks, meas+WfAug merged 4) placed manually on the three DMA
    queues in need-order; the ACT engine issues only one early DMA so
    its queue stays free for the per-tile relu accumulations; the PE
    is kept warm through the DMA lead-in with tiny N=64 matmuls so the
    real matmul stream runs at the full 2.4 GHz clock from the start.
"""

import os
import sys
import types

import numpy as np

if "/opt/trn_rl_repo" not in sys.path and not any(
    p.endswith("trn_rl_repo") for p in sys.path
):
    sys.path.insert(0, "/opt/trn_rl_repo")

B = 16384
EMB = 512
NUM_COMMANDS = 4
NUM_MEAS = 8
NCORES = 8
P = 128

MODE = os.environ.get("KERNEL_MM_MODE", "bf16")
N_WARM = int(os.environ.get("KERNEL_N_WARM", "70"))

_CACHE = {}


def _install_ntff_shim():
    """Recreate antenv.axon_hooks so trace=True works if requested."""
    if "antenv.axon_hooks" in sys.modules:
        return
    try:
        import antenv

        mod = types.ModuleType("antenv.axon_hooks")
        mod._hook = None
        mod.set_axon_ntff_profile_hook = lambda h: setattr(mod, "_hook", h)
        mod.get_axon_ntff_profile_hook = lambda: mod._hook
        sys.modules["antenv.axon_hooks"] = mod
        antenv.axon_hooks = mod
        from trn_agent_boot.trn_boot import _ntff_profile_via_ctypes

        mod.set_axon_ntff_profile_hook(
            _ntff_profile_via_ctypes("/opt/axon/libaxon_pjrt.so")
        )
    except Exception:
        pass


def _split_excess_waits(nc, max_waits=1):
    """The walrus in this container rejects instructions with more than
    one embedded sync-wait command. Waits execute in order on the
    issuing engine, so hoisting the excess onto preceding NOPs on the
    same engine is semantically identical."""
    from concourse import mybir

    n_split = 0
    for f in nc.m.functions:
        for bb in f.blocks:
            insts = list(bb.instructions)
            new_insts = []
            changed = False
            for inst in insts:
                si = inst.sync_info
                if si is not None and si.on_wait and len(si.on_wait) > max_waits:
                    waits = list(si.on_wait)
                    extra, keep = waits[:-max_waits], waits[-max_waits:]
                    while extra:
                        chunk, extra = extra[:max_waits], extra[max_waits:]
                        n_split += 1
                        nop = mybir.InstNoOp(
                            name=f"waitsplit_{n_split}_{inst.name}",
                            engine=inst.engine,
                            ins=[],
                            outs=[],
                            sync_info=mybir.SyncInfo(on_wait=chunk, on_update=[]),
                        )
                        new_insts.append(nop)
                    si.on_wait = keep
                    changed = True
                new_insts.append(inst)
            if changed:
                bb.instructions.clear()
                for i in new_insts:
                    bb.instructions.append(i)
    return n_split


def _strip_tail(nc):
    """Remove the end-of-kernel barrier/sem-reset tail.

    The runtime clears semaphores in its own exec preamble, and every
    engine's results flow into the output DMA via data-dependency
    semaphores, so the only thing that must remain is the sync-engine
    DRAIN that flushes the output DMA queue."""
    from concourse import mybir

    f = nc.m.functions[0]
    bb = f.blocks[-1]
    insts = list(bb.instructions)
    idx = None
    for i, inst in enumerate(insts):
        if isinstance(inst, mybir.InstDrain) and inst.engine == mybir.EngineType.SP:
            idx = i
            break
    if idx is None:
        return 0
    kept = insts[: idx + 1]
    drain = kept[-1]
    if drain.sync_info is not None:
        drain.sync_info.on_wait = []
    removed = len(insts) - len(kept)
    bb.instructions.clear()
    for i in kept:
        bb.instructions.append(i)
    return removed


def _np_sto_dtype(mode):
    if mode == "bf16":
        import ml_dtypes

        return ml_dtypes.bfloat16
    return np.float32


def _route(command):
    """Group sample indices by expert, pad each group to a multiple of
    8*128 and split evenly across cores.

    Returns caps [E] (rows per core per expert) and I [NCORES, R] row
    index arrays (R = sum(caps))."""
    caps = []
    parts = []  # per expert: [NCORES, cap_e] padded index array
    for e in range(NUM_COMMANDS):
        idx = np.nonzero(command == e)[0].astype(np.int64)
        n = len(idx)
        cap = int(np.ceil(n / (NCORES * P))) * P if n else 0
        caps.append(cap)
        if cap == 0:
            parts.append(np.zeros((NCORES, 0), np.int64))
            continue
        pad = NCORES * cap - n
        idx_pad = np.concatenate([idx, np.full(pad, idx[-1], np.int64)])
        parts.append(idx_pad.reshape(NCORES, cap))
    desc = sorted(range(NUM_COMMANDS), key=lambda e: -caps[e])
    # small expert first (fast DMA lead-in), small expert last (short
    # output tail)
    order = [desc[2], desc[0], desc[1], desc[3]]
    I = [np.concatenate([parts[e][k] for e in order]) for k in range(NCORES)]
    return [caps[e] for e in order], order, np.stack(I)


def _groups(nt):
    """Tile-group sizes (4-tile groups + remainder) for one expert."""
    gs = [4] * (nt // 4)
    if nt % 4:
        gs.append(nt % 4)
    return gs


def _build_program(R, caps, eorder, n_pos, mode):
    from contextlib import ExitStack

    import concourse.bass as bass
    import concourse.tile as tile
    from concourse import mybir

    f32 = mybir.dt.float32
    if mode == "bf16":
        MMD = mybir.dt.bfloat16
        STO = mybir.dt.bfloat16
    elif mode == "f32r":
        MMD = mybir.dt.float32r
        STO = f32
    else:
        MMD = f32
        STO = f32
    T = R // P
    esz = 2 if mode == "bf16" else 4
    WF = NUM_COMMANDS * EMB  # WfAug block columns in the merged mw tile

    nc = bass.Bass()
    # pre-tiled on host: every DMA is a dense 2D copy
    imgT_d = nc.declare_dram_parameter("img_pre", [P, 4 * R], MMD, isOutput=False)
    mw_d = nc.declare_dram_parameter("mw", [NUM_MEAS + 1, R + WF], MMD, isOutput=False)
    A_d = nc.declare_dram_parameter("A_proc", [P, NUM_COMMANDS * 4 * EMB], MMD, isOutput=False)
    b2tail_d = nc.declare_dram_parameter("b2tail", [P, T], f32, isOutput=False)
    outp_d = nc.declare_dram_parameter("outp", [P, 2, T], f32, isOutput=True)

    with tile.TileContext(nc) as tc:
        with ExitStack() as ctx:
            const_pool = ctx.enter_context(tc.tile_pool(name="const", bufs=1))
            w_pool = ctx.enter_context(tc.tile_pool(name="w", bufs=2))
            img_pool = ctx.enter_context(tc.tile_pool(name="img", bufs=8))
            junk_pool = ctx.enter_context(tc.tile_pool(name="junk", bufs=4))
            out_pool = ctx.enter_context(tc.tile_pool(name="out", bufs=1))
            ps_pool = ctx.enter_context(tc.tile_pool(name="ps", bufs=6, space="PSUM"))
            psw_pool = ctx.enter_context(tc.tile_pool(name="psw", bufs=1, space="PSUM"))

            # ---- DMA issue, manually placed & ordered --------------
            # scalar (ACT hwdge): only the first expert's img chunk —
            # issued before any ACTIVATE so the ACT queue stays free
            # for the per-tile relu accumulations.
            # sync (SP hwdge): A weights in processing order, the last
            # expert's img, then the per-expert output stores.
            # gpsimd (swdge): meas+WfAug replicas, b2tail, middle
            # experts' img chunks — all in need-order.
            glists = [_groups(cap // P) for cap in caps]

            img_sb = {}
            for i, cap in enumerate(caps):
                base = 4 * sum(caps[:i])
                col = 0
                for g, L in enumerate(glists[i]):
                    img_sb[i, g] = img_pool.tile(
                        [P, 4 * L * P], MMD, tag=f"img_{i}_{g}", name=f"img_{i}_{g}"
                    )
                    col += 4 * L * P

            def img_dma(eng, i, g):
                base = 4 * sum(caps[:i]) + sum(
                    4 * L * P for L in glists[i][:g]
                )
                w = img_sb[i, g].shape[-1]
                eng.dma_start(img_sb[i, g][:], imgT_d[:, base : base + w])

            A0_sb = w_pool.tile([P, 4 * EMB], MMD, tag="A0", name="A0_sb")
            A123_sb = w_pool.tile([P, 12 * EMB], MMD, tag="A123", name="A123_sb")

            # first-needed data first, one DMA per queue in flight
            img_dma(nc.scalar, 0, 0)
            nc.sync.dma_start(A0_sb[:], A_d[:, : 4 * EMB])
            mw_sb = const_pool.tile([P, R + WF], MMD, tag="mw", name="mw_sb")
            for j in range(4):
                nc.gpsimd.dma_start(
                    mw_sb[32 * j : 32 * j + NUM_MEAS + 1, :], mw_d[:]
                )
            nc.sync.dma_start(A123_sb[:], A_d[:, 4 * EMB :])
            b2tail_sb = const_pool.tile([P, T], f32, tag="b2tail", name="b2tail_sb")
            nc.gpsimd.dma_start(b2tail_sb[:], b2tail_d[:])
            for g in range(len(glists[1])):
                img_dma(nc.gpsimd, 1, g)
            for g in range(len(glists[2])):
                img_dma(nc.gpsimd, 2, g)
            for g in range(len(glists[3])):
                img_dma(nc.sync, 3, g)

            # ---- accumulators & consts -----------------------------
            zbias = const_pool.tile([P, 1], f32)
            nc.vector.memset(zbias[:], 0.0)
            p_pos = {}
            p_neg = {}
            for i, cap in enumerate(caps):
                if cap == 0:
                    continue
                tseg = cap // P
                p_pos[i] = out_pool.tile([P, tseg], f32, tag=f"pp_{i}", name=f"pp_{i}")
                p_neg[i] = out_pool.tile([P, tseg], f32, tag=f"pn_{i}", name=f"pn_{i}")
                nc.vector.memset(p_pos[i][:], 0.0)
                nc.vector.memset(p_neg[i][:], 0.0)

            # ---- PE warm-up: tiny matmuls spanning the DMA lead-in
            # so HAM unthrottles before the real stream begins -------
            warm_a = const_pool.tile([P, P], MMD, tag="warm_a", name="warm_a")
            nc.vector.memset(warm_a[:], 0.0)
            ps_w = psw_pool.tile([P, 64], f32, tag="warm_ps", name="ps_warm")
            for w in range(N_WARM):
                nc.tensor.matmul(
                    ps_w[:],
                    lhsT=warm_a[:],
                    rhs=warm_a[:, :64],
                    start=True,
                    stop=(w == N_WARM - 1),
                )
            junkw = junk_pool.tile([P, 1], STO, tag="junkw")
            nc.scalar.activation(
                junkw[:], ps_w[:, :1], mybir.ActivationFunctionType.Copy
            )

            # ---- main compute --------------------------------------
            def a_rhs(i, ko):
                if i == 0:
                    return A0_sb[:, ko * EMB : (ko + 1) * EMB]
                off = (i - 1) * 4 * EMB + ko * EMB
                return A123_sb[:, off : off + EMB]

            for i, cap in enumerate(caps):
                if cap == 0:
                    continue
                e = eorder[i]
                off = sum(caps[:i])
                nt = cap // P
                npe = n_pos[e]
                for g, L in enumerate(glists[i]):
                    ps_of = {}
                    # packed K=9 meas matmuls: up to 4 concurrent PE
                    # row-groups, one per tile in the group
                    for j in range(L):
                        r = g * 4 + j
                        psr = ps_pool.tile([P, EMB], f32, tag="h", name=f"ps_{i}_{r}")
                        ps_of[j] = psr
                        col = off + r * P
                        nc.tensor.matmul(
                            psr[:],
                            lhsT=mw_sb[32 * j : 32 * j + NUM_MEAS + 1, col : col + P],
                            rhs=mw_sb[
                                32 * j : 32 * j + NUM_MEAS + 1,
                                R + e * EMB : R + (e + 1) * EMB,
                            ],
                            start=True,
                            stop=False,
                            tile_position=(32 * j, 0),
                        )
                    for j in range(L):
                        r = g * 4 + j
                        for ko in range(4):
                            nc.tensor.matmul(
                                ps_of[j][:],
                                lhsT=img_sb[i, g][
                                    :, (ko * L + j) * P : (ko * L + j + 1) * P
                                ],
                                rhs=a_rhs(i, ko),
                                start=False,
                                stop=(ko == 3),
                            )
                        ps = ps_of[j]
                        junk = junk_pool.tile([P, EMB], STO, tag="junk")
                        if npe > 0:
                            nc.scalar.activation(
                                junk[:, :npe],
                                ps[:, :npe],
                                mybir.ActivationFunctionType.Relu,
                                bias=zbias[:],
                                accum_out=p_pos[i][:, r : r + 1],
                            )
                        if npe < EMB:
                            junk2 = junk_pool.tile([P, EMB], STO, tag="junk2")
                            nc.vector.tensor_scalar(
                                junk2[:, npe:],
                                ps[:, npe:],
                                0.0,
                                0.0,
                                mybir.AluOpType.max,
                                mybir.AluOpType.add,
                                accum_out=p_neg[i][:, r : r + 1],
                            )

                tseg = cap // P
                seg = slice(off // P, off // P + tseg)
                q = out_pool.tile([P, tseg], f32, tag=f"q_{i}", name=f"q_{i}")
                sig = out_pool.tile([P, tseg], f32, tag=f"sig_{i}", name=f"sig_{i}")
                outs = out_pool.tile(
                    [P, 2, tseg], f32, tag=f"outs_{i}", name=f"outs_{i}"
                )
                nc.vector.tensor_tensor(
                    q[:], p_pos[i][:], p_neg[i][:], mybir.AluOpType.subtract
                )
                nc.vector.tensor_add(q[:], q[:], b2tail_sb[:, seg])
                nc.scalar.activation(
                    sig[:],
                    q[:],
                    mybir.ActivationFunctionType.Sigmoid,
                    bias=zbias[:],
                )
                nc.vector.tensor_scalar_mul(outs[:, 0, :], sig[:], 50.0)
                nc.vector.tensor_scalar(
                    outs[:, 1, :],
                    q[:],
                    1.0,
                    -1.0,
                    mybir.AluOpType.min,
                    mybir.AluOpType.max,
                )
                nc.sync.dma_start(outp_d[:, :, seg], outs[:])

    _strip_tail(nc)
    _split_excess_waits(nc)
    return nc


def _prepare(inputs, mode):
    img_embs = np.asarray(inputs["img_embs"], np.float32)
    measurements = np.asarray(inputs["measurements"], np.float32)
    command = np.asarray(inputs["command"])
    W_meas = np.asarray(inputs["W_meas"], np.float32)
    b_meas = np.asarray(inputs["b_meas"], np.float32)
    W1 = np.asarray(inputs["W1"], np.float32)
    b1 = np.asarray(inputs["b1"], np.float32)
    W2 = np.asarray(inputs["W2"], np.float32)
    b2 = np.asarray(inputs["b2"], np.float32)

    sto = _np_sto_dtype(mode)
    caps, eorder, I = _route(command)
    R = int(sum(caps))

    # fold measurement path (float64 for the host-side precompute)
    W1h = W1[:, EMB:, :].astype(np.float64)
    Wf = np.einsum("md,edh->emh", W_meas.astype(np.float64), W1h)
    b_eff = np.einsum("d,edh->eh", b_meas.astype(np.float64), W1h) + b1
    A64 = W1[:, :EMB, :].astype(np.float64)

    # fold |w2[:, 0]| into the hidden columns and permute them so the
    # w2>0 columns come first: p = sum(relu(pos cols)) - sum(relu(neg
    # cols)), computed for free by the ACT/DVE accums in the relu pass.
    w2c = W2[:, :, 0].astype(np.float64)
    n_pos = []
    A_s = np.empty_like(A64)
    Wf_s = np.empty_like(Wf)
    b_eff_s = np.empty_like(b_eff)
    for e in range(NUM_COMMANDS):
        perm = np.argsort(w2c[e] <= 0, kind="stable")
        n_pos.append(int((w2c[e] > 0).sum()))
        sc = np.abs(w2c[e])[perm]
        A_s[e] = A64[e][:, perm] * sc[None, :]
        Wf_s[e] = Wf[e][:, perm] * sc[None, :]
        b_eff_s[e] = b_eff[e][perm] * sc
    WfAug = np.concatenate([Wf_s, b_eff_s[:, None, :]], axis=1).astype(sto)
    A = np.ascontiguousarray(A_s).astype(sto)  # [E, 512, 512]
    b2c = [float(x) for x in b2[:, 0]]

    T = R // P
    col_expert = np.concatenate(
        [np.full(caps[i] // P, eorder[i], np.int64) for i in range(NUM_COMMANDS)]
    )
    b2tail = np.broadcast_to(
        np.array([b2c[e] for e in col_expert], np.float32)[None, :], (P, T)
    ).copy()

    # A in processing order, k-chunk-major per expert: [P, i, ko, EMB]
    A_proc = np.ascontiguousarray(
        np.concatenate(
            [
                A[eorder[i]].reshape(4, P, EMB).transpose(1, 0, 2).reshape(P, 4 * EMB)
                for i in range(NUM_COMMANDS)
                if caps[i]
            ],
            axis=1,
        )
    )
    # WfAug block indexed by ORIGINAL expert id (device uses eorder[i])
    WfAug_flat = np.ascontiguousarray(WfAug.transpose(1, 0, 2)).reshape(
        NUM_MEAS + 1, NUM_COMMANDS * EMB
    )

    imgT = img_embs.T.astype(sto)  # [512, B] cast once
    measT = measurements.T  # [8, B]
    ones_row = np.ones((1, R), np.float32).astype(sto)
    in_maps = []
    for k in range(NCORES):
        Ik = I[k]
        imgT_k = imgT[:, Ik].reshape(4, P, R)  # [ko, p, r]
        # per expert block, per 4-tile group: [P, ko, tile, 128]
        blocks = []
        for i in range(NUM_COMMANDS):
            if not caps[i]:
                continue
            off = sum(caps[:i])
            col = off
            for L in _groups(caps[i] // P):
                blk = imgT_k[:, :, col : col + L * P]  # [4, P, L*128]
                blocks.append(blk.transpose(1, 0, 2).reshape(P, 4 * L * P))
                col += L * P
        img_pre = np.concatenate(blocks, axis=1)
        measAug_k = np.concatenate([measT[:, Ik].astype(sto), ones_row], axis=0)
        mw = np.concatenate([measAug_k, WfAug_flat], axis=1)
        in_maps.append(
            {
                "img_pre": np.ascontiguousarray(img_pre),
                "mw": np.ascontiguousarray(mw),
                "A_proc": A_proc,
                "b2tail": b2tail,
            }
        )
    return in_maps, I, R, caps, eorder, n_pos


def _run(inputs, mode=None, trace=False):
    """Returns ((angle, speed), BassKernelResults)."""
    mode = mode or MODE
    _install_ntff_shim()
    from concourse.bass_utils import run_bass_kernel_spmd

    in_maps, I, R, caps, eorder, n_pos = _prepare(inputs, mode)
    key = (R, tuple(caps), tuple(eorder), mode, tuple(n_pos))
    if key not in _CACHE:
        _CACHE[key] = _build_program(R, caps, eorder, n_pos, mode)
    nc = _CACHE[key]

    res = run_bass_kernel_spmd(
        nc, in_maps, core_ids=list(range(NCORES)), trace=trace
    )

    nb = int(np.asarray(inputs["command"]).shape[0])
    angle = np.zeros(nb, np.float32)
    speed = np.zeros(nb, np.float32)
    for k in range(NCORES):
        outp = res.results[k]["outp"]  # [128, 2, T]
        Ik = I[k]
        angle[Ik] = outp[:, 0, :].T.reshape(R)
        speed[Ik] = outp[:, 1, :].T.reshape(R)
    return (angle, speed), res


def kernel(**inputs):
    out, _ = _run(inputs)
    return out


# revision 17
# speedup vs baseline: 1.2796x; 1.2796x over previous
"""Trainium2 Bass kernel for nn_BranchedNetwork (moe_routing).

Computation (reference):
    meas_embs = measurements @ W_meas + b_meas           [B, 512]
    embs      = concat([img_embs, meas_embs], axis=1)    [B, 1024]
    h_e       = relu(embs @ W1[e] + b1[e])               per expert e
    out_e     = h_e @ W2[e] + b2[e]
    p[i]      = out[command[i], i, 0]
    angle     = sigmoid(p) * 50 ; speed = clip(p, -1, 1)

Strategy:
  * Per-sample routing on the host: samples grouped by command id, each
    group padded to a multiple of 8*128 rows and split evenly over the
    8 cores (data parallel, weights replicated; per-expert tile counts
    identical on every core so one SPMD program serves all 8).
  * Only the selected expert runs per sample (4x less compute), and
    only column 0 of W2 is needed.
  * The measurement path is folded on the host:
      h_pre = img @ W1[e][:512] + measAug @ WfAug[e]
    with measAug = [meas; 1] (K=9) so the device contraction is
    K = 512 (img) + 9.
  * |w2[:, 0]| is folded into the layer-1 weights with hidden columns
    permuted by sign of w2, so layer 2 reduces to
    p = sum(relu(pos cols)) - sum(relu(neg cols)) + b2, computed for
    free by ACT/DVE accumulators during the relu pass.
  * Device per 128-row tile: a packed K=9 meas matmul (4 tiles run
    concurrently in separate PE row-groups via tile_position) + 4
    K=128 img matmuls accumulate psum [128 rows, 512 hid]; ACT does
    relu+accum on the positive columns, DVE on the negative ones.
  * Schedule: few large DMAs (A weights as 2, img as ~7 per-tile-group
    chunks, meas+WfAug merged 4) placed manually on the three DMA
    queues in need-order; the ACT engine issues only one early DMA so
    its queue stays free for the per-tile relu accumulations; the PE
    is kept warm through the DMA lead-in with tiny N=64 matmuls so the
    real matmul stream runs at the full 2.4 GHz clock from the start.
"""

import os
import sys
import types

import numpy as np

if "/opt/trn_rl_repo" not in sys.path and not any(
    p.endswith("trn_rl_repo") for p in sys.path
):
    sys.path.insert(0, "/opt/trn_rl_repo")

B = 16384
EMB = 512
NUM_COMMANDS = 4
NUM_MEAS = 8
NCORES = 8
P = 128

MODE = os.environ.get("KERNEL_MM_MODE", "bf16")
N_WARM = int(os.environ.get("KERNEL_N_WARM", "60"))

_CACHE = {}


def _install_ntff_shim():
    """Recreate antenv.axon_hooks so trace=True works if requested."""
    if "antenv.axon_hooks" in sys.modules:
        return
    try:
        import antenv

        mod = types.ModuleType("antenv.axon_hooks")
        mod._hook = None
        mod.set_axon_ntff_profile_hook = lambda h: setattr(mod, "_hook", h)
        mod.get_axon_ntff_profile_hook = lambda: mod._hook
        sys.modules["antenv.axon_hooks"] = mod
        antenv.axon_hooks = mod
        from trn_agent_boot.trn_boot import _ntff_profile_via_ctypes

        mod.set_axon_ntff_profile_hook(
            _ntff_profile_via_ctypes("/opt/axon/libaxon_pjrt.so")
        )
    except Exception:
        pass


def _split_excess_waits(nc, max_waits=1):
    """The walrus in this container rejects instructions with more than
    one embedded sync-wait command. Waits execute in order on the
    issuing engine, so hoisting the excess onto preceding NOPs on the
    same engine is semantically identical."""
    from concourse import mybir

    n_split = 0
    for f in nc.m.functions:
        for bb in f.blocks:
            insts = list(bb.instructions)
            new_insts = []
            changed = False
            for inst in insts:
                si = inst.sync_info
                if si is not None and si.on_wait and len(si.on_wait) > max_waits:
                    waits = list(si.on_wait)
                    extra, keep = waits[:-max_waits], waits[-max_waits:]
                    while extra:
                        chunk, extra = extra[:max_waits], extra[max_waits:]
                        n_split += 1
                        nop = mybir.InstNoOp(
                            name=f"waitsplit_{n_split}_{inst.name}",
                            engine=inst.engine,
                            ins=[],
                            outs=[],
                            sync_info=mybir.SyncInfo(on_wait=chunk, on_update=[]),
                        )
                        new_insts.append(nop)
                    si.on_wait = keep
                    changed = True
                new_insts.append(inst)
            if changed:
                bb.instructions.clear()
                for i in new_insts:
                    bb.instructions.append(i)
    return n_split


def _strip_tail(nc):
    """Remove the end-of-kernel barrier/sem-reset tail.

    The runtime clears semaphores in its own exec preamble, and every
    engine's results flow into the output DMA via data-dependency
    semaphores, so the only thing that must remain is the sync-engine
    DRAIN that flushes the output DMA queue."""
    from concourse import mybir

    f = nc.m.functions[0]
    bb = f.blocks[-1]
    insts = list(bb.instructions)
    idx = None
    for i, inst in enumerate(insts):
        if isinstance(inst, mybir.InstDrain) and inst.engine == mybir.EngineType.SP:
            idx = i
            break
    if idx is None:
        return 0
    kept = insts[: idx + 1]
    drain = kept[-1]
    if drain.sync_info is not None:
        drain.sync_info.on_wait = []
    removed = len(insts) - len(kept)
    bb.instructions.clear()
    for i in kept:
        bb.instructions.append(i)
    return removed


def _np_sto_dtype(mode):
    if mode == "bf16":
        import ml_dtypes

        return ml_dtypes.bfloat16
    return np.float32


def _route(command):
    """Group sample indices by expert, pad each group to a multiple of
    8*128 and split evenly across cores.

    Returns caps [E] (rows per core per expert) and I [NCORES, R] row
    index arrays (R = sum(caps))."""
    caps = []
    parts = []  # per expert: [NCORES, cap_e] padded index array
    for e in range(NUM_COMMANDS):
        idx = np.nonzero(command == e)[0].astype(np.int64)
        n = len(idx)
        cap = int(np.ceil(n / (NCORES * P))) * P if n else 0
        caps.append(cap)
        if cap == 0:
            parts.append(np.zeros((NCORES, 0), np.int64))
            continue
        pad = NCORES * cap - n
        idx_pad = np.concatenate([idx, np.full(pad, idx[-1], np.int64)])
        parts.append(idx_pad.reshape(NCORES, cap))
    desc = sorted(range(NUM_COMMANDS), key=lambda e: -caps[e])
    # small expert first (fast DMA lead-in), small expert last (short
    # output tail)
    order = [desc[2], desc[0], desc[1], desc[3]]
    I = [np.concatenate([parts[e][k] for e in order]) for k in range(NCORES)]
    return [caps[e] for e in order], order, np.stack(I)


def _groups(nt):
    """Tile-group sizes (4-tile groups + remainder) for one expert."""
    gs = [4] * (nt // 4)
    if nt % 4:
        gs.append(nt % 4)
    return gs


def _build_program(R, caps, eorder, n_pos, mode, strip=True):
    from contextlib import ExitStack

    import concourse.bass as bass
    import concourse.tile as tile
    from concourse import mybir

    f32 = mybir.dt.float32
    if mode == "bf16":
        MMD = mybir.dt.bfloat16
        STO = mybir.dt.bfloat16
    elif mode == "f32r":
        MMD = mybir.dt.float32r
        STO = f32
    else:
        MMD = f32
        STO = f32
    T = R // P
    esz = 2 if mode == "bf16" else 4
    WF = NUM_COMMANDS * EMB  # WfAug block columns in the merged mw tile

    nc = bass.Bass()
    # pre-tiled on host: every DMA is a dense 2D copy
    imgT_d = nc.declare_dram_parameter("img_pre", [P, 4 * R], MMD, isOutput=False)
    mw_d = nc.declare_dram_parameter("mw", [NUM_MEAS + 1, R + WF], MMD, isOutput=False)
    A_d = nc.declare_dram_parameter("A_proc", [P, NUM_COMMANDS * 4 * EMB], MMD, isOutput=False)
    b2tail_d = nc.declare_dram_parameter("b2tail", [P, T], f32, isOutput=False)
    outp_d = nc.declare_dram_parameter("outp", [P, 2, T], f32, isOutput=True)

    with tile.TileContext(nc) as tc:
        with ExitStack() as ctx:
            const_pool = ctx.enter_context(tc.tile_pool(name="const", bufs=1))
            w_pool = ctx.enter_context(tc.tile_pool(name="w", bufs=1))
            img_pool = ctx.enter_context(tc.tile_pool(name="img", bufs=1))
            junk_pool = ctx.enter_context(tc.tile_pool(name="junk", bufs=4))
            out_pool = ctx.enter_context(tc.tile_pool(name="out", bufs=1))
            ps_pool = ctx.enter_context(tc.tile_pool(name="ps", bufs=8, space="PSUM"))

            # ---- DMA issue, manually placed & ordered --------------
            # Bulk transfers go only on the two HWDGE rings (sync +
            # scalar), interleaved in global need-order; the SWDGE
            # (gpsimd) ring starves when the HW rings are busy, so it
            # only carries the small meas/bias constants. The scalar
            # engine finishes all its DMA issues before its first
            # ACTIVATE so the relu accumulations are never queued
            # behind a descriptor-generation instruction.
            glists = [_groups(cap // P) for cap in caps]

            img_sb = {}
            for i, cap in enumerate(caps):
                for g, L in enumerate(glists[i]):
                    if i == 0 and g == 0:
                        # lead-in: split ko 0-1 / ko 2-3 so the first
                        # matmuls start after half the data
                        img_sb[0, 0, "a"] = img_pool.tile(
                            [P, 2 * L * P], MMD, tag="img_00a", name="img_00a"
                        )
                        img_sb[0, 0, "b"] = img_pool.tile(
                            [P, 2 * L * P], MMD, tag="img_00b", name="img_00b"
                        )
                    else:
                        img_sb[i, g] = img_pool.tile(
                            [P, 4 * L * P], MMD, tag=f"img_{i}_{g}", name=f"img_{i}_{g}"
                        )

            def img_dma(eng, i, g, half=None):
                base = 4 * sum(caps[:i]) + sum(4 * L * P for L in glists[i][:g])
                L = glists[i][g]
                if half is None:
                    eng.dma_start(
                        img_sb[i, g][:], imgT_d[:, base : base + 4 * L * P]
                    )
                else:
                    off = base + (0 if half == "a" else 2 * L * P)
                    eng.dma_start(
                        img_sb[i, g, half][:], imgT_d[:, off : off + 2 * L * P]
                    )

            A_sb = {}
            A_sb[0, "a"] = w_pool.tile([P, 2 * EMB], MMD, tag="A0a", name="A0a")
            A_sb[0, "b"] = w_pool.tile([P, 2 * EMB], MMD, tag="A0b", name="A0b")
            A_sb[1] = w_pool.tile([P, 4 * EMB], MMD, tag="A1", name="A1")
            A_sb[23] = w_pool.tile([P, 8 * EMB], MMD, tag="A23", name="A23")

            # emission order per engine = issue order = need order
            img_dma(nc.scalar, 0, 0, "a")
            nc.sync.dma_start(A_sb[0, "a"][:], A_d[:, : 2 * EMB])
            img_dma(nc.scalar, 0, 0, "b")
            nc.sync.dma_start(A_sb[0, "b"][:], A_d[:, 2 * EMB : 4 * EMB])
            # meas+WfAug replicas (9 rows at partition offsets
            # 0/32/64/96), two per HW ring
            mw_sb = const_pool.tile([P, R + WF], MMD, tag="mw", name="mw_sb")
            for j, eng in ((0, nc.sync), (1, nc.sync), (2, nc.scalar), (3, nc.scalar)):
                eng.dma_start(mw_sb[32 * j : 32 * j + NUM_MEAS + 1, :], mw_d[:])
            b2tail_sb = const_pool.tile([P, T], f32, tag="b2tail", name="b2tail_sb")
            nc.gpsimd.dma_start(b2tail_sb[:], b2tail_d[:])
            for g in range(1, len(glists[0])):
                img_dma(nc.scalar, 0, g)
            if caps[1]:
                img_dma(nc.scalar, 1, 0)
                nc.sync.dma_start(A_sb[1][:], A_d[:, 4 * EMB : 8 * EMB])
                for g in range(1, len(glists[1])):
                    img_dma(nc.scalar, 1, g)
            if caps[2] or caps[3]:
                nc.sync.dma_start(A_sb[23][:], A_d[:, 8 * EMB :])
            if caps[2]:
                for g in range(len(glists[2])):
                    img_dma(nc.scalar, 2, g)
            if caps[3]:
                for g in range(len(glists[3])):
                    img_dma(nc.sync, 3, g)

            # ---- accumulators & consts -----------------------------
            zbias = const_pool.tile([P, 1], f32)
            nc.vector.memset(zbias[:], 0.0)
            p_pos = {}
            p_neg = {}
            for i, cap in enumerate(caps):
                if cap == 0:
                    continue
                tseg = cap // P
                p_pos[i] = out_pool.tile([P, tseg], f32, tag=f"pp_{i}", name=f"pp_{i}")
                p_neg[i] = out_pool.tile([P, tseg], f32, tag=f"pn_{i}", name=f"pn_{i}")
                nc.vector.memset(p_pos[i][:], 0.0)
                nc.vector.memset(p_neg[i][:], 0.0)

            # ---- PE warm-up: tiny matmuls spanning the DMA lead-in
            # so HAM unthrottles before the real stream begins -------
            warm_a = const_pool.tile([P, P], MMD, tag="warm_a", name="warm_a")
            nc.vector.memset(warm_a[:], 0.0)
            ps_w = ps_pool.tile([P, EMB], f32, tag="h", name="ps_warm")
            for w in range(N_WARM):
                nc.tensor.matmul(
                    ps_w[:, :64],
                    lhsT=warm_a[:],
                    rhs=warm_a[:, :64],
                    start=(w == 0),
                    stop=(w == N_WARM - 1),
                )
            junkw = junk_pool.tile([P, 1], f32, tag="junkw")
            nc.vector.tensor_scalar_mul(junkw[:], ps_w[:, :1], 1.0)
            # dummy sigmoid: pull the ACT table load into the DMA
            # lead-in instead of the middle of the matmul stream
            junk_sig = junk_pool.tile([P, 1], f32, tag="junksig")
            nc.scalar.activation(
                junk_sig[:], zbias[:], mybir.ActivationFunctionType.Sigmoid
            )

            # ---- main compute --------------------------------------
            def a_rhs(i, ko):
                if i == 0:
                    return A_sb[0, "a" if ko < 2 else "b"][
                        :, (ko % 2) * EMB : (ko % 2 + 1) * EMB
                    ]
                if i == 1:
                    return A_sb[1][:, ko * EMB : (ko + 1) * EMB]
                off = (i - 2) * 4 * EMB + ko * EMB
                return A_sb[23][:, off : off + EMB]

            def img_lhsT(i, g, L, ko, j):
                if i == 0 and g == 0:
                    sb = img_sb[0, 0, "a" if ko < 2 else "b"]
                    c = ((ko % 2) * L + j) * P
                else:
                    sb = img_sb[i, g]
                    c = (ko * L + j) * P
                return sb[:, c : c + P]

            def meas_mm(i, g, L, j, start, stop):
                e = eorder[i]
                off = sum(caps[:i])
                col = off + (g * 4 + j) * P
                nc.tensor.matmul(
                    ps_of[j][:],
                    lhsT=mw_sb[32 * j : 32 * j + NUM_MEAS + 1, col : col + P],
                    rhs=mw_sb[
                        32 * j : 32 * j + NUM_MEAS + 1,
                        R + e * EMB : R + (e + 1) * EMB,
                    ],
                    start=start,
                    stop=stop,
                    tile_position=(32 * j, 0),
                )

            def accum(i, j, g, L):
                e = eorder[i]
                npe = n_pos[e]
                r = g * 4 + j
                ps = ps_of[j]
                junk = junk_pool.tile([P, EMB], STO, tag="junk")
                if npe > 0:
                    nc.scalar.activation(
                        junk[:, :npe],
                        ps[:, :npe],
                        mybir.ActivationFunctionType.Relu,
                        bias=zbias[:],
                        accum_out=p_pos[i][:, r : r + 1],
                    )
                if npe < EMB:
                    junk2 = junk_pool.tile([P, EMB], STO, tag="junk2")
                    nc.vector.tensor_scalar(
                        junk2[:, npe:],
                        ps[:, npe:],
                        0.0,
                        0.0,
                        mybir.AluOpType.max,
                        mybir.AluOpType.add,
                        accum_out=p_neg[i][:, r : r + 1],
                    )

            for i, cap in enumerate(caps):
                if cap == 0:
                    continue
                off = sum(caps[:i])
                for g, L in enumerate(glists[i]):
                    ps_of = {}
                    for j in range(L):
                        ps_of[j] = ps_pool.tile(
                            [P, EMB], f32, tag="h", name=f"ps_{i}_{g * 4 + j}"
                        )
                    if i == 0 and g == 0:
                        # lead-in group: ko-major img matmuls so the
                        # stream starts after only the ko 0-1 halves
                        # of A/img; the packed meas matmuls close the
                        # accumulation at group end (mw arrives on the
                        # slow SWDGE ring)
                        for ko in range(4):
                            for j in range(L):
                                nc.tensor.matmul(
                                    ps_of[j][:],
                                    lhsT=img_lhsT(i, g, L, ko, j),
                                    rhs=a_rhs(i, ko),
                                    start=(ko == 0),
                                    stop=False,
                                )
                        for j in range(L):
                            meas_mm(i, g, L, j, False, True)
                        for j in range(L):
                            accum(i, j, g, L)
                    else:
                        # steady state: packed meas matmuls first,
                        # then per-tile img chains with staggered
                        # ACT/DVE accumulation
                        for j in range(L):
                            meas_mm(i, g, L, j, True, False)
                        for j in range(L):
                            for ko in range(4):
                                nc.tensor.matmul(
                                    ps_of[j][:],
                                    lhsT=img_lhsT(i, g, L, ko, j),
                                    rhs=a_rhs(i, ko),
                                    start=False,
                                    stop=(ko == 3),
                                )
                            accum(i, j, g, L)

                tseg = cap // P
                seg = slice(off // P, off // P + tseg)
                q = out_pool.tile([P, tseg], f32, tag=f"q_{i}", name=f"q_{i}")
                sig = out_pool.tile([P, tseg], f32, tag=f"sig_{i}", name=f"sig_{i}")
                outs = out_pool.tile(
                    [P, 2, tseg], f32, tag=f"outs_{i}", name=f"outs_{i}"
                )
                nc.vector.tensor_tensor(
                    q[:], p_pos[i][:], p_neg[i][:], mybir.AluOpType.subtract
                )
                nc.vector.tensor_add(q[:], q[:], b2tail_sb[:, seg])
                nc.scalar.activation(
                    sig[:],
                    q[:],
                    mybir.ActivationFunctionType.Sigmoid,
                    bias=zbias[:],
                )
                nc.vector.tensor_scalar_mul(outs[:, 0, :], sig[:], 50.0)
                nc.vector.tensor_scalar(
                    outs[:, 1, :],
                    q[:],
                    1.0,
                    -1.0,
                    mybir.AluOpType.min,
                    mybir.AluOpType.max,
                )
                nc.sync.dma_start(outp_d[:, :, seg], outs[:])

    if strip:
        _strip_tail(nc)
        _split_excess_waits(nc)
    return nc


def _prepare(inputs, mode):
    img_embs = np.asarray(inputs["img_embs"], np.float32)
    measurements = np.asarray(inputs["measurements"], np.float32)
    command = np.asarray(inputs["command"])
    W_meas = np.asarray(inputs["W_meas"], np.float32)
    b_meas = np.asarray(inputs["b_meas"], np.float32)
    W1 = np.asarray(inputs["W1"], np.float32)
    b1 = np.asarray(inputs["b1"], np.float32)
    W2 = np.asarray(inputs["W2"], np.float32)
    b2 = np.asarray(inputs["b2"], np.float32)

    sto = _np_sto_dtype(mode)
    caps, eorder, I = _route(command)
    R = int(sum(caps))

    # fold measurement path (float64 for the host-side precompute)
    W1h = W1[:, EMB:, :].astype(np.float64)
    Wf = np.einsum("md,edh->emh", W_meas.astype(np.float64), W1h)
    b_eff = np.einsum("d,edh->eh", b_meas.astype(np.float64), W1h) + b1
    A64 = W1[:, :EMB, :].astype(np.float64)

    # fold |w2[:, 0]| into the hidden columns and permute them so the
    # w2>0 columns come first: p = sum(relu(pos cols)) - sum(relu(neg
    # cols)), computed for free by the ACT/DVE accums in the relu pass.
    w2c = W2[:, :, 0].astype(np.float64)
    n_pos = []
    A_s = np.empty_like(A64)
    Wf_s = np.empty_like(Wf)
    b_eff_s = np.empty_like(b_eff)
    for e in range(NUM_COMMANDS):
        perm = np.argsort(w2c[e] <= 0, kind="stable")
        n_pos.append(int((w2c[e] > 0).sum()))
        sc = np.abs(w2c[e])[perm]
        A_s[e] = A64[e][:, perm] * sc[None, :]
        Wf_s[e] = Wf[e][:, perm] * sc[None, :]
        b_eff_s[e] = b_eff[e][perm] * sc
    WfAug = np.concatenate([Wf_s, b_eff_s[:, None, :]], axis=1).astype(sto)
    A = np.ascontiguousarray(A_s).astype(sto)  # [E, 512, 512]
    b2c = [float(x) for x in b2[:, 0]]

    T = R // P
    col_expert = np.concatenate(
        [np.full(caps[i] // P, eorder[i], np.int64) for i in range(NUM_COMMANDS)]
    )
    b2tail = np.broadcast_to(
        np.array([b2c[e] for e in col_expert], np.float32)[None, :], (P, T)
    ).copy()

    # A in processing order, k-chunk-major per expert: [P, i, ko, EMB]
    A_proc = np.ascontiguousarray(
        np.concatenate(
            [
                A[eorder[i]].reshape(4, P, EMB).transpose(1, 0, 2).reshape(P, 4 * EMB)
                for i in range(NUM_COMMANDS)
                if caps[i]
            ],
            axis=1,
        )
    )
    # WfAug block indexed by ORIGINAL expert id (device uses eorder[i])
    WfAug_flat = np.ascontiguousarray(WfAug.transpose(1, 0, 2)).reshape(
        NUM_MEAS + 1, NUM_COMMANDS * EMB
    )

    imgT = img_embs.T.astype(sto)  # [512, B] cast once
    measT = measurements.T  # [8, B]
    ones_row = np.ones((1, R), np.float32).astype(sto)
    in_maps = []
    for k in range(NCORES):
        Ik = I[k]
        imgT_k = imgT[:, Ik].reshape(4, P, R)  # [ko, p, r]
        # per expert block, per 4-tile group: [P, ko, tile, 128]
        blocks = []
        for i in range(NUM_COMMANDS):
            if not caps[i]:
                continue
            off = sum(caps[:i])
            col = off
            for L in _groups(caps[i] // P):
                blk = imgT_k[:, :, col : col + L * P]  # [4, P, L*128]
                blocks.append(blk.transpose(1, 0, 2).reshape(P, 4 * L * P))
                col += L * P
        img_pre = np.concatenate(blocks, axis=1)
        measAug_k = np.concatenate([measT[:, Ik].astype(sto), ones_row], axis=0)
        mw = np.concatenate([measAug_k, WfAug_flat], axis=1)
        in_maps.append(
            {
                "img_pre": np.ascontiguousarray(img_pre),
                "mw": np.ascontiguousarray(mw),
                "A_proc": A_proc,
                "b2tail": b2tail,
            }
        )
    return in_maps, I, R, caps, eorder, n_pos


def _run(inputs, mode=None, trace=False):
    """Returns ((angle, speed), BassKernelResults)."""
    mode = mode or MODE
    _install_ntff_shim()
    from concourse.bass_utils import run_bass_kernel_spmd

    in_maps, I, R, caps, eorder, n_pos = _prepare(inputs, mode)
    key = (R, tuple(caps), tuple(eorder), mode, tuple(n_pos))
    if key not in _CACHE:
        _CACHE[key] = _build_program(R, caps, eorder, n_pos, mode)
    nc = _CACHE[key]

    res = run_bass_kernel_spmd(
        nc, in_maps, core_ids=list(range(NCORES)), trace=trace
    )

    nb = int(np.asarray(inputs["command"]).shape[0])
    angle = np.zeros(nb, np.float32)
    speed = np.zeros(nb, np.float32)
    for k in range(NCORES):
        outp = res.results[k]["outp"]  # [128, 2, T]
        Ik = I[k]
        angle[Ik] = outp[:, 0, :].T.reshape(R)
        speed[Ik] = outp[:, 1, :].T.reshape(R)
    return (angle, speed), res


def kernel(**inputs):
    out, _ = _run(inputs)
    return out
